# revision 27
# baseline (speedup 1.0000x reference)
"""Trainium2 Bass kernel for DynamicRoutingLayer.

Reference computation (the N_ITER loop is degenerate: logits do not depend on
rw, so the final rw is just softmax of the once-computed logits):
    L[b,h,n,m] = (x[b] @ W[h] @ x[b].T) * D**-0.5
    P = softmax(L, axis=-1)
    out[b]     = mean_h(P[b,h] @ x[b])

Sharding: data-parallel over B (8 batches -> 8 cores), W replicated.

Kernel per core (batch b), matmul operands in float16 (fp16's 11-bit
significand matches fp32r's PE input rounding, so accuracy is on par with
the fp32r variant, ~2e-3 rel err; but 2-byte operands get fast weight
load, halve SBUF/DMA traffic, and enable DMA-transpose):
    yT_h = (x_b @ W_h)^T        via matmul(lhsT=W_h, rhs=xT_b)  [512,1024]
    L    = yT_h^T @ xT_b        per n-tile -> PSUM [128,1024] fp32
    softmax: DVE reduce_max over 128 cols (+40 safety margin; softmax is
             shift-invariant so any c within ~80 of the true row max is
             numerically safe with fp32 e_t) -> ACT Exp(bias=-c,
             accum_out=rowsum) -> DVE reciprocal
    P_sum = sum_h P_h accumulated in fp16 via fused DVE
            scalar_tensor_tensor (out = mean_h(P_h) @ x by linearity)
    per n-tile tail: 8 fp16 PE transposes of P_sum -> one fp16 PSUM
            bank -> ACT copy to SBUF, then 8 out-matmuls accumulating
            over m into one PSUM bank.
            (A DMA-transpose variant exists behind use_dma_transpose;
            it measured within noise of PE transposes but with higher
            variance, so PE transposes are the default.)

Host-side folds: D**-0.5 into W; the 1/H head-mean into the "x" operand
(x/4) used by the out matmul.  PSUM->SBUF copies run on ACT so DVE only
carries the softmax chain.
"""

import sys

if "/opt/trn_rl_repo" not in sys.path:
    sys.path.insert(0, "/opt/trn_rl_repo")

import numpy as np

import concourse.mybir as mybir
from concourse import bacc
from concourse.bass import ts
from concourse.tile import TileContext
from concourse.bass_utils import run_bass_kernel_spmd

B, N, D = 8, 1024, 512
H = 4
P = 128
NT = N // P       # 8 n-tiles (query rows)
MT = N // P       # 8 m-tiles (key rows)
KT = D // P       # 4 contraction tiles
NCH = N // 512    # 2 chunks of 512 along the N (m) free axis
F32 = mybir.dt.float32
F16 = mybir.dt.float16


def _dedup_ldweights(nc):
    """Remove InstLdweights whose weights AP is identical to the previous
    weight load on PE with only weight-preserving instructions (matmuls,
    event semaphores, nops, drains) in between.  The PE array keeps its
    stationary operand across matmuls, so the reload is pure overhead
    (~P/1.2 ns each).  Any sync carried by a removed load is merged into
    the next matmul."""
    removed = 0
    KEEP = ("InstMatmult", "InstEventSemaphore", "InstNop", "InstDrain")
    for blk in nc.main_func.blocks:
        insts = blk.instructions
        prev_sig = None
        to_remove = []
        for idx, inst in enumerate(insts):
            if inst.engine != mybir.EngineType.PE:
                continue
            nm = type(inst).__name__
            if nm == "InstLdweights":
                a = inst.ins[0]
                if a.regs_read():
                    prev_sig = None
                    continue
                sig = (
                    a.concise(),
                    a.offset,
                    inst.perf_mode,
                    inst.is_transpose,
                    inst.tile_position,
                )
                if sig == prev_sig:
                    to_remove.append(idx)
                else:
                    prev_sig = sig
            elif nm not in KEEP:
                prev_sig = None
        for idx in reversed(to_remove):
            inst = insts[idx]
            si = inst.sync_info
            if si is not None and (len(si.on_wait) > 0 or len(si.on_update) > 0):
                j = idx + 1
                while j < len(insts) and not (
                    insts[j].engine == mybir.EngineType.PE
                    and type(insts[j]).__name__ == "InstMatmult"
                ):
                    j += 1
                assert j < len(insts), "removed ldweights with sync but no next matmul"
                tgt = insts[j]
                tsi = tgt.sync_info
                if tsi is None:
                    tgt.sync_info = mybir.SyncInfo(
                        on_wait=list(si.on_wait), on_update=list(si.on_update)
                    )
                else:
                    tgt.sync_info = mybir.SyncInfo(
                        on_wait=list(si.on_wait) + list(tsi.on_wait),
                        on_update=list(tsi.on_update) + list(si.on_update),
                    )
            del insts[idx]
            removed += 1
    return removed


def build_kernel(
    reps=1,
    tail_h=1,
    use_dma_transpose=False,
    k_outer=False,
    do_compile=True,
    dedup_ldw=False,
):
    nc = bacc.Bacc("TRN2", target_bir_lowering=False)

    x_d = nc.dram_tensor("x", [N, D], F16, kind="ExternalInput")
    xt_d = nc.dram_tensor("xT", [D, N], F16, kind="ExternalInput")
    w_d = nc.dram_tensor("W", [H, D, D], F16, kind="ExternalInput")
    o_d = nc.dram_tensor("out", [N, D], F32, kind="ExternalOutput")

    o_tiled = o_d.rearrange("(t p) d -> t p d", p=P)

    from contextlib import ExitStack

    with TileContext(nc) as tc, ExitStack() as stack:
        if reps > 1:
            stack.enter_context(
                tc.For_i(
                    0,
                    reps,
                    1,
                    hint_engines=(
                        mybir.EngineType.PE,
                        mybir.EngineType.Activation,
                        mybir.EngineType.DVE,
                        mybir.EngineType.Pool,
                        mybir.EngineType.SP,
                    ),
                )
            )
        from concourse.masks import make_identity

        with (
            tc.tile_pool(name="const", bufs=1) as const,
            tc.tile_pool(name="ypool", bufs=1) as ypool,
            tc.tile_pool(name="psum_big", bufs=3, space="PSUM") as psum_big,
            tc.tile_pool(
                name="psum_o", bufs=(2 if use_dma_transpose else 1), space="PSUM"
            ) as psum_o,
            tc.tile_pool(name="psum_t", bufs=1, space="PSUM") as psum_t,
            tc.tile_pool(name="stat", bufs=4) as stat,
            tc.tile_pool(name="epool", bufs=3) as epool,
            tc.tile_pool(name="apool", bufs=3) as apool,
            tc.tile_pool(name="ptpool", bufs=3) as ptpool,
            tc.tile_pool(name="outpool", bufs=3) as outpool,
        ):
            if not use_dma_transpose:
                identity_f32 = const.tile([P, P], F32)
                make_identity(nc, identity_f32)
                identity = const.tile([P, P], F16)
                nc.vector.tensor_copy(identity, identity_f32)
                # PE clock warmup: dependency-free matmuls run during the
                # initial DMA fill (PE would otherwise idle ~3us), so the
                # tensor engine reaches full clock (ramp needs ~3us of
                # continuous busy) before the first real matmul issues.
                # fp32 identity: ready right after make_identity (no wait on
                # the fp16 copy), and at 4 cycles/row each matmul burns 4x
                # the ramp time, so fewer instructions are needed.
                warm_ps = psum_o.tile([P, D], F32, name="po")
                for _ in range(7):
                    nc.tensor.matmul(
                        warm_ps[:, 0:P],
                        lhsT=identity_f32,
                        rhs=identity_f32,
                        start=True,
                        stop=True,
                    )
            # load order: the first Y matmul needs only W[h0,k0] + xT[k0],
            # so those two small DMAs go first; x_nat is not needed until
            # the first pipeline tail, so it loads last.
            xt_sb = const.tile([P, KT, N], F16)   # [p, k-tile, n]
            xt_re = xt_d.rearrange("(k p) n -> k p n", p=P)
            w_sb = const.tile([P, H, KT, D], F16)  # [p, h, k-tile, e]
            w_re = w_d.rearrange("h (k p) e -> h p k e", p=P)
            nc.sync.dma_start(out=w_sb[:, 0, 0], in_=w_re[0, :, 0])
            nc.sync.dma_start(out=xt_sb[:, 0], in_=xt_re[0])
            for k in range(1, KT):
                nc.sync.dma_start(out=w_sb[:, 0, k], in_=w_re[0, :, k])
                nc.sync.dma_start(out=xt_sb[:, k], in_=xt_re[k])
            for h in range(1, H):
                nc.sync.dma_start(out=w_sb[:, h], in_=w_re[h])
            x_nat = const.tile([P, MT, D], F16)   # [p, m-tile, d]
            nc.sync.dma_start(
                out=x_nat, in_=x_d.rearrange("(t p) d -> p t d", p=P)
            )

            # yT[h] = (x @ W_h)^T, stored [p, h, e-tile, n].  k-outer /
            # chunk-inner so consecutive matmul pairs share the same lhsT.
            yt_sb = ypool.tile([P, H, KT, N], F16)
            for h in range(H):
                for e in range(KT):
                    ps = psum_big.tile([P, N], F32, tag="big")
                    if k_outer:
                        for k in range(KT):
                            for nch in range(NCH):
                                nc.tensor.matmul(
                                    ps[:, ts(nch, 512)],
                                    lhsT=w_sb[:, h, k, ts(e, P)],
                                    rhs=xt_sb[:, k, ts(nch, 512)],
                                    start=(k == 0),
                                    stop=(k == KT - 1),
                                )
                    else:
                        for nch in range(NCH):
                            for k in range(KT):
                                nc.tensor.matmul(
                                    ps[:, ts(nch, 512)],
                                    lhsT=w_sb[:, h, k, ts(e, P)],
                                    rhs=xt_sb[:, k, ts(nch, 512)],
                                    start=(k == 0),
                                    stop=(k == KT - 1),
                                )
                    nc.scalar.copy(yt_sb[:, h, e], ps)

            # main loop: per n-tile, accumulate P_sum = sum_h P_h in fp16 on
            # DVE (out = mean_h(P_h) @ x by linearity), then one
            # DMA-transpose + out-matmul tail per n-tile.  Tails deferred so
            # the softmax chain latency is covered by PE work.
            pending = []

            def emit_tail(nt, pacc):
                # half-granular: transposes/copies for columns 0:512 only
                # depend on the first half of the final stt, so the last
                # tile's tail pipelines against its own softmax instead of
                # serializing after it.
                pt = ptpool.tile([P, MT, P], F16)
                if use_dma_transpose:
                    nc.sync.dma_start_transpose(out=pt, in_=pacc)
                else:
                    pt_ps = psum_t.tile([P, N], F16, name="pt_ps")
                    for half in range(2):
                        for q in range(MT // 2):
                            mt = half * (MT // 2) + q
                            nc.tensor.transpose(
                                pt_ps[:, ts(mt, P)], pacc[:, ts(mt, P)], identity
                            )
                        # DVE: 2x rate on the fp16 data and lower
                        # per-instruction overhead than ACT, which is busy
                        # with exps in the S phase
                        nc.vector.tensor_copy(
                            pt[:, half * (MT // 2) : (half + 1) * (MT // 2), :],
                            pt_ps[:, ts(half, 512)],
                        )
                po = psum_o.tile([P, D], F32, name="po")
                for mt in range(MT):
                    nc.tensor.matmul(
                        po,
                        lhsT=pt[:, mt, :],
                        rhs=x_nat[:, mt, :],
                        start=(mt == 0),
                        stop=(mt == MT - 1),
                    )
                osb = outpool.tile([P, D], F32)
                # DVE (idle at the tail) instead of ACT, which is still
                # draining the pt copies
                nc.vector.tensor_copy(osb, po)
                nc.sync.dma_start(out=o_tiled[nt], in_=osb)

            pacc = None
            for nt in range(NT):
                for h in range(H):
                    psl = psum_big.tile([P, N], F32, tag="big")
                    if k_outer:
                        for k in range(KT):
                            for nch in range(NCH):
                                nc.tensor.matmul(
                                    psl[:, ts(nch, 512)],
                                    lhsT=yt_sb[:, h, k, ts(nt, P)],
                                    rhs=xt_sb[:, k, ts(nch, 512)],
                                    start=(k == 0),
                                    stop=(k == KT - 1),
                                )
                    else:
                        for nch in range(NCH):
                            for k in range(KT):
                                nc.tensor.matmul(
                                    psl[:, ts(nch, 512)],
                                    lhsT=yt_sb[:, h, k, ts(nt, P)],
                                    rhs=xt_sb[:, k, ts(nch, 512)],
                                    start=(k == 0),
                                    stop=(k == KT - 1),
                                )
                    negmax = stat.tile([P, 1], F32)
                    nc.vector.reduce_max(
                        negmax, psl[:, 0:P], axis=mybir.AxisListType.X, negate=True
                    )
                    nc.vector.tensor_scalar_add(negmax, negmax, -40.0)
                    e_t = epool.tile([P, N], F32)
                    ssum = stat.tile([P, 1], F32)
                    nc.scalar.activation(
                        out=e_t,
                        in_=psl,
                        func=mybir.ActivationFunctionType.Exp,
                        bias=negmax,
                        scale=1.0,
                        accum_out=ssum,
                    )
                    rinv = stat.tile([P, 1], F32)
                    nc.vector.reciprocal(rinv, ssum)
                    if h == 0:
                        pacc = apool.tile([P, N], F16, name="pacc")
                        nc.vector.tensor_scalar_mul(pacc, e_t, rinv)
                    elif h == H - 1:
                        # final accumulate in two column halves so the tail's
                        # first-half transposes can start while the second
                        # half is still on DVE
                        for half in range(2):
                            nc.vector.scalar_tensor_tensor(
                                out=pacc[:, ts(half, 512)],
                                in0=e_t[:, ts(half, 512)],
                                scalar=rinv,
                                in1=pacc[:, ts(half, 512)],
                                op0=mybir.AluOpType.mult,
                                op1=mybir.AluOpType.add,
                            )
                    else:
                        # pacc += e_t * rinv, fused (fp32 e_t in, fp16 out)
                        nc.vector.scalar_tensor_tensor(
                            out=pacc,
                            in0=e_t,
                            scalar=rinv,
                            in1=pacc,
                            op0=mybir.AluOpType.mult,
                            op1=mybir.AluOpType.add,
                        )
                    if h == H - 1:
                        pending.append((nt, pacc))
                    if pending and h == tail_h:
                        emit_tail(*pending.pop(0))
            for p in pending:
                emit_tail(*p)

    if dedup_ldw:
        n = _dedup_ldweights(nc)
        assert n > 0, "dedup_ldw found nothing to remove"
    if do_compile:
        nc.compile()
    return nc


_NC_CACHE = None


def make_in_maps(x, W):
    x = np.asarray(x, dtype=np.float32)
    W = np.asarray(W, dtype=np.float32)
    scale = np.float32(D ** -0.5)
    w16 = np.ascontiguousarray((W * scale).astype(np.float16))

    in_maps = []
    for b in range(B):
        xb = np.ascontiguousarray(x[b])
        in_maps.append(
            {
                # 1/H head-mean folded into the out-matmul operand
                "x": np.ascontiguousarray((xb * np.float32(1.0 / H)).astype(np.float16)),
                "xT": np.ascontiguousarray(xb.T.astype(np.float16)),
                "W": w16,
            }
        )
    return in_maps


def kernel(x, W):
    global _NC_CACHE
    if _NC_CACHE is None:
        _NC_CACHE = build_kernel()
    nc = _NC_CACHE

    in_maps = make_in_maps(x, W)
    res = run_bass_kernel_spmd(nc, in_maps, core_ids=list(range(B)))
    out = np.stack([res.results[b]["out"] for b in range(B)], axis=0)
    return out
